# revision 36
# baseline (speedup 1.0000x reference)
"""Trainium2 Bass kernel for nn_DynamicFiltering (optimized).

Computation (per batch b):
  y  = LeakyReLU(conv2d(x_t, w1, b1), 0.2)        per frame t
  ker = conv2d(y, w2, b2)                          (t, 9, h, w)
  ker = ker - mean_K(ker) + 1/45                   per-pixel over K = 45
  out[c,h,w] = sum_{t,k1,k2} x_edge[c,t,h+k1-1,w+k2-1] * ker[t,k1,k2][h,w]

Sharding: 8 cores = 2 batches x 4 H-slabs of 32 rows.

Structure:
  - all conv matmuls in bf16 (1 cycle/row on the PE vs ~2 for fp32),
    frames packed in pairs on the 128-partition contraction dim with
    block-diagonal weights: 3 conv passes (f01, f23, f4)
  - conv2 output goes through the DMA xbar transpose + a DVE repack into
    kt2[q, ti, r] (r innermost) so the per-pixel kernel broadcast runs
    the DVE in 2x bf16 mode
  - dynamic-filter products: bf16 tensor_tensor; ~2/3 on DVE (2x mode) and
    1/3 on the Pool engine; pairwise tree accumulation on DVE
  - pass p's DVE filter work is emitted as a deferred backlog drained
    between pass p+1's leaky ops, so the in-order DVE queue never blocks
    the conv pipeline
  - the dj column shift of the patches is baked into partition-shifted
    copies of x (made by on-device DMA, edge-replicated), so there is a
    single accumulator and no post-transpose merge
  - final combine (pass sums + pass-2 products + c*S) is accumulated on
    the PE into PSUM with a bf16 identity, then transposed out
  - normalization term: out += c * S with c = 1/45 - mean(ker),
    S = 3x3 box sum of U (U = sum of frames) via partition-shifted U
"""

import numpy as np

DIM = 64
T = 5
H = 128
W = 128
SLAB = 32          # output rows per core
NCORES = 8
GH = 36            # conv grid rows: slab + 2*2 halo
GW = 130           # conv grid cols: W + 2
FR = 34            # filter rows: slab + 2 halo
NPASS = 3          # frame pairs: (0,1), (2,3), (4,-)

_PROGRAM_CACHE = {}


def _build_program():
    import concourse.bacc as bacc
    import concourse.mybir as mybir
    from concourse.tile import TileContext

    f32 = mybir.dt.float32
    bf16 = mybir.dt.bfloat16
    u16 = mybir.dt.uint16
    Act = mybir.ActivationFunctionType
    Alu = mybir.AluOpType
    Ax = mybir.AxisListType

    nc = bacc.Bacc("TRN2", debug=False)

    xc_d = nc.dram_tensor("xc", [NPASS, 128, GH, GW], bf16, kind="ExternalInput").ap()
    xt_d = nc.dram_tensor("xt", [3, W, T, DIM, FR], bf16, kind="ExternalInput").ap()
    w1a_d = nc.dram_tensor("w1a", [128, 9, 128], bf16, kind="ExternalInput").ap()
    w1b_d = nc.dram_tensor("w1b", [128, 9, 128], bf16, kind="ExternalInput").ap()
    w2a_d = nc.dram_tensor("w2a", [128, 9, 18], bf16, kind="ExternalInput").ap()
    w2b_d = nc.dram_tensor("w2b", [128, 9, 18], bf16, kind="ExternalInput").ap()
    b1_d = nc.dram_tensor("b1r", [128, 1], f32, kind="ExternalInput").ap()
    b1s_d = nc.dram_tensor("b1s", [128, 1], f32, kind="ExternalInput").ap()
    b2a_d = nc.dram_tensor("b2a", [18, 1], f32, kind="ExternalInput").ap()
    b2b_d = nc.dram_tensor("b2b", [18, 1], f32, kind="ExternalInput").ap()
    ym_d = nc.dram_tensor("ymask", [128, 2], f32, kind="ExternalInput").ap()
    id_d = nc.dram_tensor("ident", [128, 128], f32, kind="ExternalInput").ap()
    idb_d = nc.dram_tensor("identb", [128, 128], bf16, kind="ExternalInput").ap()
    out_d = nc.dram_tensor("out", [DIM, SLAB, W], f32, kind="ExternalOutput").ap()

    # 4 rows per chunk: a matmul's PSUM output must fit one 2KB bank
    C1_CHUNKS = [(1 + 4 * i, 4) for i in range(8)] + [(33, 2)]
    C2_CHUNKS = [(2 + 4 * i, 4) for i in range(8)]
    N_POOL = [5, 5, 4]   # filter products per pass computed on Pool

    with TileContext(nc) as tc:
        with (
            tc.tile_pool(name="consts", bufs=1) as cpool,
            tc.tile_pool(name="xtp", bufs=1) as xtp,
            tc.tile_pool(name="xcp", bufs=2) as xcp,
            tc.tile_pool(name="yp", bufs=2) as yp,
            tc.tile_pool(name="y0p", bufs=2) as y0p,
            tc.tile_pool(name="kst", bufs=1) as kstp,
            tc.tile_pool(name="ktp", bufs=1) as ktp,
            tc.tile_pool(name="up", bufs=1) as up,
            tc.tile_pool(name="tp", bufs=6) as tp,
            tc.tile_pool(name="ppp", bufs=5) as ppp,
            tc.tile_pool(name="obp", bufs=2) as obp,
        ):
            # pass-0 conv input first: the xt DMA below is 2.8MB and the
            # convs must not wait behind it
            xc0 = xcp.tile([128, GH, GW], bf16, tag="xc")
            nc.sync.dma_start(out=xc0, in_=xc_d[0])

            w1a_sb = cpool.tile([128, 9, 128], bf16)
            nc.sync.dma_start(out=w1a_sb, in_=w1a_d)
            w1b_sb = cpool.tile([128, 9, 128], bf16)
            nc.sync.dma_start(out=w1b_sb, in_=w1b_d)
            w2a_sb = cpool.tile([128, 9, 18], bf16)
            nc.sync.dma_start(out=w2a_sb, in_=w2a_d)
            w2b_sb = cpool.tile([128, 9, 18], bf16)
            nc.sync.dma_start(out=w2b_sb, in_=w2b_d)
            b1_sb = cpool.tile([128, 1], f32)
            nc.sync.dma_start(out=b1_sb, in_=b1_d)
            b1s_sb = cpool.tile([128, 1], f32)
            nc.sync.dma_start(out=b1s_sb, in_=b1s_d)
            b2a_sb = cpool.tile([18, 1], f32)
            nc.sync.dma_start(out=b2a_sb, in_=b2a_d)
            b2b_sb = cpool.tile([18, 1], f32)
            nc.sync.dma_start(out=b2b_sb, in_=b2b_d)
            ym_sb = cpool.tile([128, 2], f32)
            nc.sync.dma_start(out=ym_sb, in_=ym_d)
            id_sb = cpool.tile([128, 128], f32)
            nc.sync.dma_start(out=id_sb, in_=id_d)
            idb_sb = cpool.tile([128, 128], bf16)
            nc.sync.dma_start(out=idb_sb, in_=idb_d)

            # shifted x copies for the filter: xt[dj][q] = x[.., q + dj - 1]
            # (edge-replicated at q=0/127 by the host).  All three come from
            # DRAM: an on-device partition-shift is a SBUF-SBUF DMA that runs
            # on a single DMA engine (~124us for 2.8MB) — far too slow.
            xt = []
            for dj in range(3):
                t_ = xtp.tile([W, T, DIM, FR], bf16, name=f"xt{dj}")
                nc.gpsimd.dma_start(out=t_, in_=xt_d[dj])
                xt.append(t_)
            xt1 = xt[1]

            ker_st = kstp.tile([32, SLAB, W], bf16)
            nc.gpsimd.memset(ker_st.bitcast(u16), 0)
            kt2 = [ktp.tile([W, 32, SLAB], bf16, name=f"kt2_{p}")
                   for p in range(NPASS)]
            pass_sums = [ktp.tile([W, DIM, SLAB], bf16, name=f"psum{p}")
                         for p in range(2)]

            u_c = up.tile([W, DIM, FR], bf16, name="u_c")
            u_m1 = up.tile([W, DIM, FR], bf16, name="u_m1")
            u_p1 = up.tile([W, DIM, FR], bf16, name="u_p1")
            sv = up.tile([W, DIM, SLAB], bf16, name="sv")

            # ----- deferred DVE work (drained between leaky ops) -----
            backlog = []

            def drain(n):
                for _ in range(min(n, len(backlog))):
                    backlog.pop(0)()

            def _u_chain():
                th = []
                th.append(lambda: nc.vector.tensor_tensor(
                    u_c, xt1[:, 0], xt1[:, 1], Alu.add))
                for t_i in (2, 3, 4):
                    th.append(lambda t_i=t_i: nc.vector.tensor_tensor(
                        u_c, u_c, xt1[:, t_i], Alu.add))
                # partition-shifted copies; edges replicate
                def shifts():
                    nc.gpsimd.dma_start(out=u_m1[1:128], in_=u_c[0:127])
                    nc.gpsimd.dma_start(out=u_m1[0:1], in_=u_c[0:1])
                    nc.gpsimd.dma_start(out=u_p1[0:127], in_=u_c[1:128])
                    nc.gpsimd.dma_start(out=u_p1[127:128], in_=u_c[127:128])
                th.append(shifts)
                th.append(lambda: nc.vector.tensor_tensor(
                    u_m1, u_c, u_m1, Alu.add))
                th.append(lambda: nc.vector.tensor_tensor(
                    u_m1, u_m1, u_p1, Alu.add))
                th.append(lambda: nc.vector.tensor_tensor(
                    sv, u_m1[:, :, 0:SLAB], u_m1[:, :, 1:SLAB + 1], Alu.add))
                th.append(lambda: nc.vector.tensor_tensor(
                    sv, sv, u_m1[:, :, 2:SLAB + 2], Alu.add))
                return th

            def _prod_into(tile, t_i, fi, di, dj, p, eng):
                ti = fi * 9 + 3 * di + dj
                kb = kt2[p][:, ti, :].unsqueeze(1).broadcast_to((W, DIM, SLAB))
                eng.tensor_tensor(tile, xt[dj][:, t_i, :, di:di + SLAB],
                                  kb, Alu.mult)

            def _pass_filter(p):
                """Emit Pool products now; return DVE thunks for pass p
                (p < 2 only — pass 2 is handled at the tail)."""
                frames = [2 * p, 2 * p + 1]
                terms = [(t_i, fi, di, dj)
                         for fi, t_i in enumerate(frames)
                         for di in range(3) for dj in range(3)]
                n_pool = N_POOL[p]
                pool_terms = terms[len(terms) - n_pool:]
                dve_terms = terms[:len(terms) - n_pool]
                pool_tiles = []
                for (t_i, fi, di, dj) in pool_terms:
                    tl = ppp.tile([W, DIM, SLAB], bf16, tag="pp")
                    _prod_into(tl, t_i, fi, di, dj, p, nc.gpsimd)
                    pool_tiles.append(tl)
                # counter tree over DVE terms; the Pool products fold in via
                # a sequential chain interleaved mid-stream (frees their
                # slots early; they become ready one-by-one on Pool anyway)
                state = {"stack": [], "pchain": None, "pleft": list(pool_tiles)}

                def mk_prod(t_i, fi, di, dj):
                    def run():
                        tl = tp.tile([W, DIM, SLAB], bf16, tag="ts")
                        _prod_into(tl, t_i, fi, di, dj, p, nc.vector)
                        cur, sz = tl, 1
                        st = state["stack"]
                        while st and st[-1][1] == sz:
                            prev, _ = st.pop()
                            nxt = tp.tile([W, DIM, SLAB], bf16, tag="ts")
                            nc.vector.tensor_tensor(nxt, prev, cur, Alu.add)
                            cur, sz = nxt, sz * 2
                        st.append((cur, sz))
                    return run

                def mk_pfold():
                    def run():
                        if not state["pleft"]:
                            return
                        tl = state["pleft"].pop(0)
                        if state["pchain"] is None:
                            state["pchain"] = tl
                        else:
                            nxt = tp.tile([W, DIM, SLAB], bf16, tag="ts")
                            nc.vector.tensor_tensor(nxt, state["pchain"], tl,
                                                    Alu.add)
                            state["pchain"] = nxt
                    return run

                th = []
                di_ = 0
                for (t_i, fi, di, dj) in dve_terms:
                    th.append(mk_prod(t_i, fi, di, dj))
                    di_ += 1
                    if di_ % 3 == 0:
                        th.append(mk_pfold())

                def fold():
                    while state["pleft"]:
                        mk_pfold()()
                    items = [t for t, _ in state["stack"]]
                    if state["pchain"] is not None:
                        items.append(state["pchain"])
                    while len(items) > 2:
                        a_ = items.pop(0)
                        b_ = items.pop(0)
                        nxt = tp.tile([W, DIM, SLAB], bf16, tag="ts")
                        nc.vector.tensor_tensor(nxt, a_, b_, Alu.add)
                        items.append(nxt)
                    nc.vector.tensor_tensor(pass_sums[p], items[0], items[1],
                                            Alu.add)
                th.append(fold)
                return th

            with (
                tc.tile_pool(name="ps1", bufs=2, space="PSUM") as ps1p,
                tc.tile_pool(name="ps2", bufs=2, space="PSUM") as ps2p,
            ):
                for p in range(NPASS):
                    w1_sb = w1a_sb if p < 2 else w1b_sb
                    w2_sb = w2a_sb if p < 2 else w2b_sb
                    b2_sb = b2a_sb if p < 2 else b2b_sb

                    if p == 0:
                        xc_f = xc0
                    else:
                        xc_f = xcp.tile([128, GH, GW], bf16, tag="xc")
                        nc.sync.dma_start(out=xc_f, in_=xc_d[p])
                    y_f = yp.tile([128, GH, GW], bf16, tag="y")
                    # zero y edge cols via Act (zero-input write)
                    nc.scalar.activation(y_f[:, 1:35, 0:1], y_f[:, 1:35, 0:1],
                                         Act.Copy, scale=0.0)
                    nc.scalar.activation(y_f[:, 1:35, 129:130],
                                         y_f[:, 1:35, 129:130],
                                         Act.Copy, scale=0.0)

                    # conv1 + leaky relu
                    for g0, nr in C1_CHUNKS:
                        ps = ps1p.tile([128, 4, W], f32, tag="ps1")
                        for idx in range(9):
                            di, dj = divmod(idx, 3)
                            rhs = xc_f[:, g0 + di - 1:g0 + di - 1 + nr, dj:dj + W]
                            nc.tensor.matmul(
                                ps[:, :nr, :],
                                lhsT=w1_sb[:, idx, :],
                                rhs=rhs,
                                start=(idx == 0),
                                stop=(idx == 8),
                            )
                        y0 = y0p.tile([128, 4, W], bf16, tag="y0")
                        a0 = y0p.tile([128, 4, W], bf16, tag="a0")
                        nc.scalar.activation(y0[:, :nr], ps[:, :nr], Act.Identity,
                                             bias=b1_sb, scale=1.0)
                        # a0 = (2/3)|y0| via Abs((2/3) ps + (2/3) b1)
                        nc.scalar.activation(a0[:, :nr], ps[:, :nr], Act.Abs,
                                             bias=b1s_sb, scale=2.0 / 3.0)
                        # y = y0 + (2/3)|y0| == leaky(y0)/0.6; w2 carries 0.6
                        nc.vector.tensor_tensor(y_f[:, g0:g0 + nr, 1:129],
                                                y0[:, :nr], a0[:, :nr], Alu.add)
                        # keep the DVE queue shallow while leaky ops (which
                        # gate conv2) are still being enqueued
                        if p > 0:
                            drain(1)

                    # conv2 zero-pads rows outside the image: kill y halo rows
                    nc.scalar.activation(y_f[:, 1:2, 1:129], y_f[:, 1:2, 1:129],
                                         Act.Copy, scale=ym_sb[:, 0:1])
                    nc.scalar.activation(y_f[:, 34:35, 1:129], y_f[:, 34:35, 1:129],
                                         Act.Copy, scale=ym_sb[:, 1:2])

                    # conv2 -> ker_st[ti, r, q]; grid row = 2 + r
                    for g0, nr in C2_CHUNKS:
                        ps2 = ps2p.tile([18, 4, W], f32, tag="ps2")
                        for idx in range(9):
                            di, dj = divmod(idx, 3)
                            rhs = y_f[:, g0 + di - 1:g0 + di - 1 + nr, dj:dj + W]
                            nc.tensor.matmul(
                                ps2,
                                lhsT=w2_sb[:, idx, :],
                                rhs=rhs,
                                start=(idx == 0),
                                stop=(idx == 8),
                            )
                        nc.scalar.activation(ker_st[0:18, g0 - 2:g0 - 2 + nr, :],
                                             ps2, Act.Identity, bias=b2_sb,
                                             scale=1.0)
                        drain(2 if p == 0 else 3)

                    # DMA xbar transpose: ker_st[ti, (r, q)] -> ktA[q, r, ti]
                    # then DVE repack to kt2[q, ti, r] (r innermost)
                    ktA = kstp.tile([W, SLAB, 32], bf16, tag="ktA")
                    nc.sync.dma_start_transpose(
                        out=ktA, in_=ker_st.rearrange("ti r q -> ti (r q)"))
                    nc.vector.tensor_copy(
                        kt2[p], ktA.rearrange("q r ti -> q ti r"))

                    if p == 0:
                        backlog.extend(_u_chain())
                    if p < 2:
                        backlog.extend(_pass_filter(p))
                    else:
                        # pass-2 Pool products start now (Pool is free)
                        pass2_pool = []
                        for (di, dj) in [(2, 0), (2, 1), (2, 2), (1, 2)]:
                            tl = ppp.tile([W, DIM, SLAB], bf16, tag="pp")
                            _prod_into(tl, 4, 0, di, dj, 2, nc.gpsimd)
                            pass2_pool.append(tl)

            drain(len(backlog))

            # --- normalization coefficient c = 1/45 - mean(ker) ---
            r0_ = ktp.tile([W, SLAB], f32, name="r0")
            r1_ = ktp.tile([W, SLAB], f32, name="r1")
            r2_ = ktp.tile([W, SLAB], f32, name="r2")
            nc.vector.tensor_reduce(r0_, kt2[0].rearrange("q ti r -> q r ti")
                                    [:, :, 0:18], axis=Ax.X, op=Alu.add)
            nc.vector.tensor_reduce(r1_, kt2[1].rearrange("q ti r -> q r ti")
                                    [:, :, 0:18], axis=Ax.X, op=Alu.add)
            nc.vector.tensor_reduce(r2_, kt2[2].rearrange("q ti r -> q r ti")
                                    [:, :, 0:9], axis=Ax.X, op=Alu.add)
            nc.vector.tensor_tensor(r0_, r0_, r1_, Alu.add)
            nc.vector.tensor_tensor(r0_, r0_, r2_, Alu.add)
            c_sb = ktp.tile([W, SLAB], f32, name="c_sb")
            nc.vector.tensor_scalar(c_sb, r0_, -1.0 / 45.0, 1.0 / 45.0,
                                    Alu.mult, Alu.add)
            c_bf = ktp.tile([W, SLAB], bf16, name="c_bf")
            nc.vector.tensor_copy(c_bf, c_sb)

            # out += c * S  (joins the PE accumulation)
            cs_prod = tp.tile([W, DIM, SLAB], bf16, tag="ts")
            cb = c_bf.unsqueeze(1).broadcast_to((W, DIM, SLAB))
            nc.vector.tensor_tensor(cs_prod, sv, cb, Alu.mult)

            # --- final accumulation on the PE (bf16 identity matmuls into
            # PSUM): pass-2 DVE products incrementally, then the pool
            # products, c*S, and the two pass sums ---
            total = ktp.tile([W, DIM, SLAB], f32, name="total")
            tot_flat = total.rearrange("q c r -> q (c r)")
            out_rc = out_d.rearrange("c r w -> (c r) w")
            p2_dve = [(0, 0), (0, 1), (0, 2), (1, 0), (1, 1)]
            with (
                tc.tile_pool(name="acc", bufs=1, space="PSUM") as accp,
                tc.tile_pool(name="pso", bufs=2, space="PSUM") as psop,
            ):
                acc = accp.tile([W, DIM * SLAB], f32)
                nt = len(p2_dve) + len(pass2_pool) + 3

                def accum(ti_i, tl):
                    fl = tl.rearrange("q c r -> q (c r)")
                    for cc in range(4):
                        sl = slice(512 * cc, 512 * (cc + 1))
                        nc.tensor.matmul(
                            acc[:, sl], lhsT=idb_sb, rhs=fl[:, sl],
                            start=(ti_i == 0), stop=(ti_i == nt - 1))

                ti_i = 0
                for (di, dj) in p2_dve:
                    tl = tp.tile([W, DIM, SLAB], bf16, tag="ts")
                    _prod_into(tl, 4, 0, di, dj, 2, nc.vector)
                    accum(ti_i, tl)
                    ti_i += 1
                for tl in pass2_pool + [cs_prod, pass_sums[0], pass_sums[1]]:
                    accum(ti_i, tl)
                    ti_i += 1
                for cc in range(4):
                    sl = slice(512 * cc, 512 * (cc + 1))
                    nc.scalar.activation(tot_flat[:, sl], acc[:, sl],
                                         Act.Copy, scale=1.0)
                    for oc in range(4 * cc, 4 * cc + 4):
                        so = slice(128 * oc, 128 * (oc + 1))
                        pst = psop.tile([128, 128], f32, tag="pso")
                        nc.tensor.transpose(pst, tot_flat[:, so], id_sb)
                        ob = obp.tile([128, 128], f32, tag="ob")
                        nc.scalar.activation(ob, pst, Act.Copy, scale=1.0)
                        nc.sync.dma_start(out=out_rc[so], in_=ob)

    return nc


def _get_program():
    if "nc" not in _PROGRAM_CACHE:
        nc = _build_program()
        nc.finalize()
        _PROGRAM_CACHE["nc"] = nc
    return _PROGRAM_CACHE["nc"]


def _host_prep(x, w1, b1, w2, b2):
    """Build the 8 per-core input maps from full inputs."""
    import ml_dtypes
    bf = ml_dtypes.bfloat16

    x = np.asarray(x, dtype=np.float32)
    w1 = np.asarray(w1, dtype=np.float32)
    b1 = np.asarray(b1, dtype=np.float32)
    w2 = np.asarray(w2, dtype=np.float32)
    b2 = np.asarray(b2, dtype=np.float32)

    # block-diagonal packed weights (2 frames per conv pass)
    w1t = w1.transpose(1, 2, 3, 0).reshape(DIM, 9, DIM)  # [ci, tap, o]
    # y is stored as leaky/0.6 (the Abs trick); fold the 0.6 into w2
    w2t = 0.6 * w2.transpose(1, 2, 3, 0).reshape(DIM, 9, 9)
    w1a = np.zeros((128, 9, 128), np.float32)
    w1a[0:64, :, 0:64] = w1t
    w1a[64:128, :, 64:128] = w1t
    w1b = np.zeros((128, 9, 128), np.float32)
    w1b[0:64, :, 0:64] = w1t
    w2a = np.zeros((128, 9, 18), np.float32)
    w2a[0:64, :, 0:9] = w2t
    w2a[64:128, :, 9:18] = w2t
    w2b = np.zeros((128, 9, 18), np.float32)
    w2b[0:64, :, 0:9] = w2t

    b1r = np.concatenate([b1, b1]).reshape(128, 1).astype(np.float32)
    b1s = ((2.0 / 3.0) * b1r).astype(np.float32)
    b2a = np.concatenate([b2, b2]).reshape(18, 1).astype(np.float32)
    b2b = np.concatenate([b2, 0 * b2]).reshape(18, 1).astype(np.float32)
    ident = np.eye(128, dtype=np.float32)
    identb = np.eye(128, dtype=np.float32).astype(bf)

    w1a = w1a.astype(bf)
    w1b = w1b.astype(bf)
    w2a = w2a.astype(bf)
    w2b = w2b.astype(bf)

    in_maps = []
    for core in range(NCORES):
        b, s = divmod(core, 4)
        r0 = s * SLAB
        # conv input per pass: frames (2p, 2p+1) stacked on 128 partitions,
        # rows r0-2 .. r0+33 zero padded, cols -1..128 zero padded
        xc = np.zeros((NPASS, 128, GH, GW), np.float32)
        lo = max(0, r0 - 2)
        hi = min(H, r0 + 34)
        for p in range(NPASS):
            for f in range(2):
                t = 2 * p + f
                if t >= T:
                    continue
                xc[p, f * 64:(f + 1) * 64,
                   lo - (r0 - 2):hi - (r0 - 2), 1:129] = x[b, :, t, lo:hi, :]
        # filter input, pixel-partition, 3 dj-shifted copies:
        # xt[dj][q, t, c, rr] = x[b, c, t, clip(r0-1+rr), clip(q+dj-1)]
        rows = np.clip(np.arange(r0 - 1, r0 + 33), 0, H - 1)
        base = x[b][:, :, rows, :]            # (c, t, 34, w)
        xt = np.empty((3, W, T, DIM, FR), np.float32)
        for dj in range(3):
            cols = np.clip(np.arange(dj - 1, W + dj - 1), 0, W - 1)
            xt[dj] = base[:, :, :, cols].transpose(3, 1, 0, 2)
        # conv2 zero-pad mask for the y halo rows (grid rows 1 and 34)
        ymask = np.ones((128, 2), np.float32)
        if s == 0:
            ymask[:, 0] = 0.0
        if s == 3:
            ymask[:, 1] = 0.0
        in_maps.append({
            "xc": xc.astype(bf), "xt": xt.astype(bf),
            "w1a": w1a, "w1b": w1b, "w2a": w2a, "w2b": w2b,
            "b1r": b1r, "b1s": b1s, "b2a": b2a, "b2b": b2b,
            "ymask": ymask, "ident": ident, "identb": identb,
        })
    return in_maps


def kernel(x, w1, b1, w2, b2):
    from concourse.bass_utils import run_bass_kernel_spmd

    nc = _get_program()
    in_maps = _host_prep(x, w1, b1, w2, b2)
    res = run_bass_kernel_spmd(nc, in_maps, list(range(NCORES)))
    out = np.zeros((2, DIM, H, W), dtype=np.float32)
    for core in range(NCORES):
        b, s = divmod(core, 4)
        out[b, :, s * SLAB:(s + 1) * SLAB, :] = res.results[core]["out"]
    return out


# revision 42
# speedup vs baseline: 1.1293x; 1.1293x over previous
"""Trainium2 Bass kernel for nn_DynamicFiltering (optimized).

Computation (per batch b):
  y  = LeakyReLU(conv2d(x_t, w1, b1), 0.2)        per frame t
  ker = conv2d(y, w2, b2)                          (t, 9, h, w)
  ker = ker - mean_K(ker) + 1/45                   per-pixel over K = 45
  out[c,h,w] = sum_{t,k1,k2} x_edge[c,t,h+k1-1,w+k2-1] * ker[t,k1,k2][h,w]

Sharding: 8 cores = 2 batches x 4 H-slabs of 32 rows.

Key structure vs the fp32 baseline:
  - all conv matmuls in bf16 (fp32 runs ~2-3 cycles/row on the PE, bf16 1)
  - frames packed in pairs on the 128-partition contraction dim with
    block-diagonal weights: 3 conv passes (f01, f23, f4) instead of 5
  - LeakyReLU = one Act bias-add + one Pool scalar_tensor_tensor max(0.2y,y)
  - conv2 output transposed to pixel-partition layout by the DMA xbar
    (dma_start_transpose), writing [q, ti, r] with r innermost so the
    per-pixel kernel broadcast runs the DVE in 2x bf16 mode
  - dynamic-filter products on DVE as bf16 tensor_tensor (2x), pairwise
    tree accumulation, a few products per pass on the Pool engine
  - the dj column shift of the patches is baked into 3 host-prepared
    shifted copies of x (edge-replicated), so there is a single fp32
    accumulator and no post-transpose merge
  - normalization term: out += c * S with c = 1/45 - mean(ker),
    S = 3x3 box sum of U (U = sum of frames), built from partition-shifted
    copies of U via SBUF-SBUF DMA
"""

import numpy as np

DIM = 64
T = 5
H = 128
W = 128
SLAB = 32          # output rows per core
NCORES = 8
GH = 36            # conv grid rows: slab + 2*2 halo
GW = 130           # conv grid cols: W + 2
FR = 34            # filter rows: slab + 2 halo
NPASS = 3          # frame pairs: (0,1), (2,3), (4,-)

_PROGRAM_CACHE = {}


def _build_program():
    import concourse.bacc as bacc
    import concourse.mybir as mybir
    from concourse.tile import TileContext

    f32 = mybir.dt.float32
    bf16 = mybir.dt.bfloat16
    u16 = mybir.dt.uint16
    Act = mybir.ActivationFunctionType
    Alu = mybir.AluOpType
    Ax = mybir.AxisListType

    nc = bacc.Bacc("TRN2", debug=False)

    xc_d = nc.dram_tensor("xc", [NPASS, 128, GH, GW], bf16, kind="ExternalInput").ap()
    xt_d = nc.dram_tensor("xt", [3, W, T, DIM, FR], bf16, kind="ExternalInput").ap()
    w1a_d = nc.dram_tensor("w1a", [128, 9, 128], bf16, kind="ExternalInput").ap()
    w1b_d = nc.dram_tensor("w1b", [128, 9, 128], bf16, kind="ExternalInput").ap()
    w2a_d = nc.dram_tensor("w2a", [128, 9, 18], bf16, kind="ExternalInput").ap()
    w2b_d = nc.dram_tensor("w2b", [128, 9, 18], bf16, kind="ExternalInput").ap()
    b1_d = nc.dram_tensor("b1r", [128, 1], f32, kind="ExternalInput").ap()
    b1s_d = nc.dram_tensor("b1s", [128, 1], f32, kind="ExternalInput").ap()
    b2a_d = nc.dram_tensor("b2a", [18, 1], f32, kind="ExternalInput").ap()
    b2b_d = nc.dram_tensor("b2b", [18, 1], f32, kind="ExternalInput").ap()
    ym_d = nc.dram_tensor("ymask", [128, 2], f32, kind="ExternalInput").ap()
    id_d = nc.dram_tensor("ident", [128, 128], f32, kind="ExternalInput").ap()
    out_d = nc.dram_tensor("out", [DIM, SLAB, W], f32, kind="ExternalOutput").ap()

    # conv1 chunks: grid out rows 1..34;  conv2 chunks: grid out rows 2..33
    # (4 rows per chunk: a matmul's PSUM output must fit one 2KB bank)
    C1_CHUNKS = [(1 + 4 * i, 4) for i in range(8)] + [(33, 2)]
    C2_CHUNKS = [(2 + 4 * i, 4) for i in range(8)]

    with TileContext(nc) as tc:
        with (
            tc.tile_pool(name="consts", bufs=1) as cpool,
            tc.tile_pool(name="xtp", bufs=1) as xtp,
            tc.tile_pool(name="xcp", bufs=2) as xcp,
            tc.tile_pool(name="yp", bufs=2) as yp,
            tc.tile_pool(name="y0p", bufs=2) as y0p,
            tc.tile_pool(name="kst", bufs=1) as kstp,
            tc.tile_pool(name="ktp", bufs=1) as ktp,
            tc.tile_pool(name="up", bufs=1) as up,
            tc.tile_pool(name="tp", bufs=7) as tp,
            tc.tile_pool(name="obp", bufs=3) as obp,
        ):
            w1a_sb = cpool.tile([128, 9, 128], bf16)
            nc.sync.dma_start(out=w1a_sb, in_=w1a_d)
            w1b_sb = cpool.tile([128, 9, 128], bf16)
            nc.sync.dma_start(out=w1b_sb, in_=w1b_d)
            w2a_sb = cpool.tile([128, 9, 18], bf16)
            nc.sync.dma_start(out=w2a_sb, in_=w2a_d)
            w2b_sb = cpool.tile([128, 9, 18], bf16)
            nc.sync.dma_start(out=w2b_sb, in_=w2b_d)
            b1_sb = cpool.tile([128, 1], f32)
            nc.sync.dma_start(out=b1_sb, in_=b1_d)
            b1s_sb = cpool.tile([128, 1], f32)
            nc.sync.dma_start(out=b1s_sb, in_=b1s_d)
            b2a_sb = cpool.tile([18, 1], f32)
            nc.sync.dma_start(out=b2a_sb, in_=b2a_d)
            b2b_sb = cpool.tile([18, 1], f32)
            nc.sync.dma_start(out=b2b_sb, in_=b2b_d)
            ym_sb = cpool.tile([128, 2], f32)
            nc.sync.dma_start(out=ym_sb, in_=ym_d)
            id_sb = cpool.tile([128, 128], f32)
            nc.sync.dma_start(out=id_sb, in_=id_d)

            # pass-0 conv input first: the xt DMAs below are 8.4MB and the
            # convs must not wait behind them
            xc0 = xcp.tile([128, GH, GW], bf16, tag="xc")
            nc.sync.dma_start(out=xc0, in_=xc_d[0])

            # shifted x copies for the filter: xt[dj][q] = x[.., q + dj - 1]
            # (edge-replicated at q=0/127 by the host)
            xt = []
            for dj in range(3):
                t_ = xtp.tile([W, T, DIM, FR], bf16, name=f"xt{dj}")
                nc.gpsimd.dma_start(out=t_, in_=xt_d[dj])
                xt.append(t_)

            # conv2 -> kernel staging (bf16, pixel cols innermost) and the
            # transposed per-pixel kernels kt2[p][q, ti, r] (r innermost)
            ker_st = kstp.tile([32, SLAB, W], bf16)
            nc.gpsimd.memset(ker_st.bitcast(u16), 0)
            kt2 = [ktp.tile([W, 32, SLAB], bf16, name=f"kt2_{p}")
                   for p in range(NPASS)]
            ktap = kstp  # transpose staging shares the kst pool
            pass_sums = [ktp.tile([W, DIM, SLAB], bf16, name=f"psum{p}")
                         for p in range(NPASS)]

            # --- S path: U = sum_t x_t, U3 = sum_dj U_dj, Sv = 3-row box ---
            u_c = up.tile([W, DIM, FR], bf16, name="u_c")
            u_m1 = up.tile([W, DIM, FR], bf16, name="u_m1")
            u_p1 = up.tile([W, DIM, FR], bf16, name="u_p1")
            sv = up.tile([W, DIM, SLAB], bf16, name="sv")
            # U chain on DVE: it is idle until the first products (~35us),
            # while Pool must be free for the leaky adds from pass 0 on
            nc.vector.tensor_tensor(u_c, xt[1][:, 0], xt[1][:, 1], Alu.add)
            for t_i in (2, 3, 4):
                nc.vector.tensor_tensor(u_c, u_c, xt[1][:, t_i], Alu.add)
            # partition-shifted copies (DMA is exempt from the start-partition
            # restriction); edges replicate
            nc.gpsimd.dma_start(out=u_m1[1:128], in_=u_c[0:127])
            nc.gpsimd.dma_start(out=u_m1[0:1], in_=u_c[0:1])
            nc.gpsimd.dma_start(out=u_p1[0:127], in_=u_c[1:128])
            nc.gpsimd.dma_start(out=u_p1[127:128], in_=u_c[127:128])
            # u_m1 becomes U3 = U_c + U_m1 + U_p1 in place
            nc.vector.tensor_tensor(u_m1, u_c, u_m1, Alu.add)
            nc.vector.tensor_tensor(u_m1, u_m1, u_p1, Alu.add)
            nc.vector.tensor_tensor(sv, u_m1[:, :, 0:SLAB],
                                    u_m1[:, :, 1:SLAB + 1], Alu.add)
            nc.vector.tensor_tensor(sv, sv, u_m1[:, :, 2:SLAB + 2], Alu.add)

            with (
                tc.tile_pool(name="ps1", bufs=3, space="PSUM") as ps1p,
                tc.tile_pool(name="ps2", bufs=3, space="PSUM") as ps2p,
            ):
                for p in range(NPASS):
                    w1_sb = w1a_sb if p < 2 else w1b_sb
                    w2_sb = w2a_sb if p < 2 else w2b_sb
                    b2_sb = b2a_sb if p < 2 else b2b_sb

                    if p == 0:
                        xc_f = xc0
                    else:
                        xc_f = xcp.tile([128, GH, GW], bf16, tag="xc")
                        nc.sync.dma_start(out=xc_f, in_=xc_d[p])
                    y_f = yp.tile([128, GH, GW], bf16, tag="y")
                    nc.gpsimd.memset(y_f[:, 1:35, 0:1].bitcast(u16), 0)
                    nc.gpsimd.memset(y_f[:, 1:35, 129:130].bitcast(u16), 0)

                    # conv1 + leaky relu
                    for g0, nr in C1_CHUNKS:
                        ps = ps1p.tile([128, 4, W], f32, tag="ps1")
                        for idx in range(9):
                            di, dj = divmod(idx, 3)
                            rhs = xc_f[:, g0 + di - 1:g0 + di - 1 + nr, dj:dj + W]
                            nc.tensor.matmul(
                                ps[:, :nr, :],
                                lhsT=w1_sb[:, idx, :],
                                rhs=rhs,
                                start=(idx == 0),
                                stop=(idx == 8),
                            )
                        y0 = y0p.tile([128, 4, W], bf16, tag="y0")
                        a0 = y0p.tile([128, 4, W], bf16, tag="a0")
                        nc.scalar.activation(y0[:, :nr], ps[:, :nr], Act.Identity,
                                             bias=b1_sb, scale=1.0)
                        # a0 = (2/3)|y0| via Abs((2/3) ps + (2/3) b1)
                        nc.scalar.activation(a0[:, :nr], ps[:, :nr], Act.Abs,
                                             bias=b1s_sb, scale=2.0 / 3.0)
                        # y = y0 + (2/3)|y0| == leaky(y0)/0.6; w2 carries the
                        # 0.6.  On Pool: DVE must stay free for the products.
                        nc.gpsimd.tensor_tensor(y_f[:, g0:g0 + nr, 1:129],
                                                y0[:, :nr], a0[:, :nr], Alu.add)

                    # conv2 zero-pads rows outside the image: kill y halo rows
                    nc.scalar.activation(y_f[:, 1:2, 1:129], y_f[:, 1:2, 1:129],
                                         Act.Copy, scale=ym_sb[:, 0:1])
                    nc.scalar.activation(y_f[:, 34:35, 1:129], y_f[:, 34:35, 1:129],
                                         Act.Copy, scale=ym_sb[:, 1:2])

                    # conv2 -> ker_st[ti, r, q]; grid row = 2 + r
                    for g0, nr in C2_CHUNKS:
                        ps2 = ps2p.tile([18, 4, W], f32, tag="ps2")
                        for idx in range(9):
                            di, dj = divmod(idx, 3)
                            rhs = y_f[:, g0 + di - 1:g0 + di - 1 + nr, dj:dj + W]
                            nc.tensor.matmul(
                                ps2,
                                lhsT=w2_sb[:, idx, :],
                                rhs=rhs,
                                start=(idx == 0),
                                stop=(idx == 8),
                            )
                        nc.scalar.activation(ker_st[0:18, g0 - 2:g0 - 2 + nr, :],
                                             ps2, Act.Identity, bias=b2_sb,
                                             scale=1.0)

                    # DMA xbar transpose: ker_st[ti, (r, q)] -> ktA[q, r, ti]
                    # (DMA needs a contiguous last dim), then DVE repack to
                    # kt2[q, ti, r] so the product broadcast has r innermost
                    ktA = ktap.tile([W, SLAB, 32], bf16, tag="ktA")
                    nc.sync.dma_start_transpose(
                        out=ktA, in_=ker_st.rearrange("ti r q -> ti (r q)"))
                    nc.vector.tensor_copy(
                        kt2[p], ktA.rearrange("q r ti -> q ti r"))

                    # dynamic-filter products for this pass's frames
                    frames = [2 * p, 2 * p + 1] if p < 2 else [4]
                    terms = [(t_i, fi, di, dj)
                             for fi, t_i in enumerate(frames)
                             for di in range(3) for dj in range(3)]
                    n_pool = 0 if p < 2 else 3
                    stack = []
                    for i, (t_i, fi, di, dj) in enumerate(terms):
                        ti = fi * 9 + 3 * di + dj
                        kb = kt2[p][:, ti, :].unsqueeze(1)\
                            .broadcast_to((W, DIM, SLAB))
                        xs = xt[dj][:, t_i, :, di:di + SLAB]
                        prod = tp.tile([W, DIM, SLAB], bf16, tag="ts")
                        eng = nc.gpsimd if i >= len(terms) - n_pool else nc.vector
                        eng.tensor_tensor(prod, xs, kb, Alu.mult)
                        cur, sz = prod, 1
                        while stack and stack[-1][1] == sz:
                            prev, _ = stack.pop()
                            nxt = tp.tile([W, DIM, SLAB], bf16, tag="ts")
                            nc.vector.tensor_tensor(nxt, prev, cur, Alu.add)
                            cur, sz = nxt, sz * 2
                        stack.append((cur, sz))
                    while len(stack) > 2:
                        b_, _ = stack.pop()
                        a_, _ = stack.pop()
                        nxt = tp.tile([W, DIM, SLAB], bf16, tag="ts")
                        nc.vector.tensor_tensor(nxt, a_, b_, Alu.add)
                        stack.append((nxt, 0))
                    b_, _ = stack.pop()
                    a_, _ = stack.pop()
                    nc.vector.tensor_tensor(pass_sums[p], a_, b_, Alu.add)

            # --- normalization coefficient c = 1/45 - mean(ker) ---
            r0_ = ktp.tile([W, SLAB], f32, name="r0")
            r1_ = ktp.tile([W, SLAB], f32, name="r1")
            r2_ = ktp.tile([W, SLAB], f32, name="r2")
            nc.vector.tensor_reduce(r0_, kt2[0].rearrange("q ti r -> q r ti")
                                    [:, :, 0:18], axis=Ax.X, op=Alu.add)
            nc.vector.tensor_reduce(r1_, kt2[1].rearrange("q ti r -> q r ti")
                                    [:, :, 0:18], axis=Ax.X, op=Alu.add)
            nc.vector.tensor_reduce(r2_, kt2[2].rearrange("q ti r -> q r ti")
                                    [:, :, 0:9], axis=Ax.X, op=Alu.add)
            nc.vector.tensor_tensor(r0_, r0_, r1_, Alu.add)
            nc.vector.tensor_tensor(r0_, r0_, r2_, Alu.add)
            c_sb = ktp.tile([W, SLAB], f32, name="c_sb")
            nc.vector.tensor_scalar(c_sb, r0_, -1.0 / 45.0, 1.0 / 45.0,
                                    Alu.mult, Alu.add)
            c_bf = ktp.tile([W, SLAB], bf16, name="c_bf")
            nc.vector.tensor_copy(c_bf, c_sb)

            # out += c * S  (joins the final combine)
            cs_prod = tp.tile([W, DIM, SLAB], bf16, tag="ts")
            cb = c_bf.unsqueeze(1).broadcast_to((W, DIM, SLAB))
            nc.vector.tensor_tensor(cs_prod, sv, cb, Alu.mult)

            # final combine in 4 column-quarters so the output transposes can
            # start as soon as each quarter is ready
            total = ktp.tile([W, DIM, SLAB], f32, name="total")
            t1 = ktp.tile([W, DIM, SLAB], f32, name="t1")
            tot_flat = total.rearrange("q c r -> q (c r)")
            out_rc = out_d.rearrange("c r w -> (c r) w")
            with tc.tile_pool(name="pso", bufs=2, space="PSUM") as psop:
                for g in range(4):
                    cg = slice(16 * g, 16 * (g + 1))
                    nc.vector.tensor_tensor(t1[:, cg], pass_sums[0][:, cg],
                                            pass_sums[1][:, cg], Alu.add)
                    nc.vector.tensor_tensor(total[:, cg], pass_sums[2][:, cg],
                                            cs_prod[:, cg], Alu.add)
                    nc.vector.tensor_tensor(total[:, cg], total[:, cg],
                                            t1[:, cg], Alu.add)
                    for oc in range(4 * g, 4 * g + 4):
                        sl = slice(128 * oc, 128 * (oc + 1))
                        pst = psop.tile([128, 128], f32, tag="pso")
                        nc.tensor.transpose(pst, tot_flat[:, sl], id_sb)
                        ob = obp.tile([128, 128], f32, tag="ob")
                        nc.scalar.activation(ob, pst, Act.Copy, scale=1.0)
                        nc.sync.dma_start(out=out_rc[sl], in_=ob)

    return nc


def _get_program():
    if "nc" not in _PROGRAM_CACHE:
        nc = _build_program()
        nc.finalize()
        _PROGRAM_CACHE["nc"] = nc
    return _PROGRAM_CACHE["nc"]


def _host_prep(x, w1, b1, w2, b2):
    """Build the 8 per-core input maps from full inputs."""
    import ml_dtypes
    bf = ml_dtypes.bfloat16

    x = np.asarray(x, dtype=np.float32)
    w1 = np.asarray(w1, dtype=np.float32)
    b1 = np.asarray(b1, dtype=np.float32)
    w2 = np.asarray(w2, dtype=np.float32)
    b2 = np.asarray(b2, dtype=np.float32)

    # block-diagonal packed weights (2 frames per conv pass)
    w1t = w1.transpose(1, 2, 3, 0).reshape(DIM, 9, DIM)  # [ci, tap, o]
    # y is stored as leaky/0.6 (the Abs trick); fold the 0.6 into w2
    w2t = 0.6 * w2.transpose(1, 2, 3, 0).reshape(DIM, 9, 9)
    w1a = np.zeros((128, 9, 128), np.float32)
    w1a[0:64, :, 0:64] = w1t
    w1a[64:128, :, 64:128] = w1t
    w1b = np.zeros((128, 9, 128), np.float32)
    w1b[0:64, :, 0:64] = w1t
    w2a = np.zeros((128, 9, 18), np.float32)
    w2a[0:64, :, 0:9] = w2t
    w2a[64:128, :, 9:18] = w2t
    w2b = np.zeros((128, 9, 18), np.float32)
    w2b[0:64, :, 0:9] = w2t

    b1r = np.concatenate([b1, b1]).reshape(128, 1).astype(np.float32)
    b1s = ((2.0 / 3.0) * b1r).astype(np.float32)
    b2a = np.concatenate([b2, b2]).reshape(18, 1).astype(np.float32)
    b2b = np.concatenate([b2, 0 * b2]).reshape(18, 1).astype(np.float32)
    ident = np.eye(128, dtype=np.float32)

    w1a = w1a.astype(bf)
    w1b = w1b.astype(bf)
    w2a = w2a.astype(bf)
    w2b = w2b.astype(bf)

    in_maps = []
    for core in range(NCORES):
        b, s = divmod(core, 4)
        r0 = s * SLAB
        # conv input per pass: frames (2p, 2p+1) stacked on 128 partitions,
        # rows r0-2 .. r0+33 zero padded, cols -1..128 zero padded
        xc = np.zeros((NPASS, 128, GH, GW), np.float32)
        lo = max(0, r0 - 2)
        hi = min(H, r0 + 34)
        for p in range(NPASS):
            for f in range(2):
                t = 2 * p + f
                if t >= T:
                    continue
                xc[p, f * 64:(f + 1) * 64,
                   lo - (r0 - 2):hi - (r0 - 2), 1:129] = x[b, :, t, lo:hi, :]
        # filter input, pixel-partition, 3 dj-shifted copies:
        # xt[dj][q, t, c, rr] = x[b, c, t, clip(r0-1+rr), clip(q+dj-1)]
        rows = np.clip(np.arange(r0 - 1, r0 + 33), 0, H - 1)
        base = x[b][:, :, rows, :]            # (c, t, 34, w)
        xt = np.empty((3, W, T, DIM, FR), np.float32)
        for dj in range(3):
            cols = np.clip(np.arange(dj - 1, W + dj - 1), 0, W - 1)
            xt[dj] = base[:, :, :, cols].transpose(3, 1, 0, 2)
        # conv2 zero-pad mask for the y halo rows (grid rows 1 and 34)
        ymask = np.ones((128, 2), np.float32)
        if s == 0:
            ymask[:, 0] = 0.0
        if s == 3:
            ymask[:, 1] = 0.0
        in_maps.append({
            "xc": xc.astype(bf), "xt": xt.astype(bf),
            "w1a": w1a, "w1b": w1b, "w2a": w2a, "w2b": w2b,
            "b1r": b1r, "b1s": b1s, "b2a": b2a, "b2b": b2b,
            "ymask": ymask, "ident": ident,
        })
    return in_maps


def kernel(x, w1, b1, w2, b2):
    from concourse.bass_utils import run_bass_kernel_spmd

    nc = _get_program()
    in_maps = _host_prep(x, w1, b1, w2, b2)
    res = run_bass_kernel_spmd(nc, in_maps, list(range(NCORES)))
    out = np.zeros((2, DIM, H, W), dtype=np.float32)
    for core in range(NCORES):
        b, s = divmod(core, 4)
        out[b, :, s * SLAB:(s + 1) * SLAB, :] = res.results[core]["out"]
    return out
